# revision 37
# baseline (speedup 1.0000x reference)
"""Haar DWT (2x2 stride-2 block decomposition) on 8 Trainium2 NeuronCores.

Input x: (32, 3, 512, 512) f32. Outputs (ll, lh, hl, hh): each (32, 3, 256, 256).

Sharding: pure data parallel over the batch dim — 4 images per core, viewed as
12 channel images of 512x512 per core.

The 2e-2 rel-err gate admits fp16 end to end (measured rel-err ~6e-4), so the
host stages the input to HBM as fp16 during sharding: HBM traffic per core is
6.29 MB in + 6.29 MB out instead of the naive 12.6 + 12.6. Channels are staged
in pairs interleaved per row ([pair, t, p, c, w]) so each DMA descriptor still
moves a contiguous 2 KB run into a partition line.

On-chip, both butterfly stages run on the TensorEngine via PSUM
accumulation — no DVE tensor-tensor work at all:

  P1 =  W @ x_even + W @ x_odd   ->  [ll (p<64) ; lh (p>=64)]
  P2 = -W @ x_even + W @ x_odd   ->  [hl ; hh]

with fp16 weights/moving operands (1 col/cycle, 4x fast weight load) and the
even/odd split taken from stride-2 access patterns. Each 2-bank PSUM group is
drained by two parallel fp32->fp16 copies (ACT and DVE halves) straight into
the store tile. Loads ride the Sync HWDGE ring, stores the ACT ring.
"""

import sys

import numpy as np

if "/opt/trn_rl_repo" not in sys.path:
    sys.path.insert(0, "/opt/trn_rl_repo")

from concourse import bacc, bass, mybir
from concourse import tile
from concourse.bass_utils import run_bass_kernel_spmd

N_CORES = 8
B, C, H, W = 32, 3, 512, 512
BPC = B // N_CORES  # images per core
NCH = BPC * C  # channel images per core (12)
NPR = NCH // 2  # channel pairs per core (6)
P = 128  # SBUF partitions
NT = H // P  # 128-row tiles per channel (4)
HW_OUT = H // 2  # 256

_CACHE = {}


def _butterfly_weights():
    """w[0] = vertical butterfly W (sum rows to p<64, diff rows to p>=64);
    w[1] = -W (for the hl/hh accumulation group's even-column pass)."""
    w = np.zeros((2, P, P), dtype=np.float32)
    for m in range(64):
        w[0, 2 * m, m] = 0.5
        w[0, 2 * m + 1, m] = 0.5
        w[0, 2 * m, 64 + m] = -0.5
        w[0, 2 * m + 1, 64 + m] = 0.5
    w[1] = -w[0]
    return w


def _build():
    nc = bacc.Bacc("TRN2", target_bir_lowering=False, debug=False)
    f32 = mybir.dt.float32
    f16 = mybir.dt.float16
    # x staged as fp16 [pair, tile, row-in-tile, channel-in-pair, W]
    x = nc.dram_tensor("x", [NPR, NT, P, 2, W], f16, kind="ExternalInput")
    w = nc.dram_tensor("w", [2, P, P], f16, kind="ExternalInput")
    # out[pair, p, c, g, t, j]: g=0: [ll (p<64) | lh (p>=64)], g=1: [hl | hh];
    # output image row r = 64*t + (p mod 64). Pair-major so one store covers
    # both channels with a contiguous 8 KB run per partition line.
    out = nc.dram_tensor(
        "out", [NPR, P, 2, 2, NT, HW_OUT], f16, kind="ExternalOutput"
    )
    xa = x.ap()
    oa = out.ap()
    with tile.TileContext(nc) as tc:
        with (
            tc.tile_pool(name="p", bufs=5) as pool,
            tc.tile_pool(name="o", bufs=4) as opool,
            tc.tile_pool(name="w", bufs=1) as wpool,
            tc.tile_pool(name="ps", bufs=4, space=bass.MemorySpace.PSUM) as psum,
        ):
            wt = wpool.tile([P, 2, P], f16)
            for pr in range(NPR):
                xin = pool.tile([P, NT, 2, W], f16)
                if pr == 0 or pr == NPR - 1:
                    # split the first load (matmuls start early) and the last
                    # load (the tail pair's matmuls aren't gated on the full
                    # 1 MiB transfer). For pair 0, order the queue by what
                    # gates the first matmul group: tiles 0-1, weights, 2-3.
                    for t in range(2):
                        nc.sync.dma_start(out=xin[:, t], in_=xa[pr, t])
                    if pr == 0:
                        nc.sync.dma_start(out=wt[:], in_=w.ap().transpose([1, 0, 2]))
                    for t in range(2, NT):
                        nc.sync.dma_start(out=xin[:, t], in_=xa[pr, t])
                elif pr == 1:
                    # the scalar HWDGE ring is idle until the first store
                    # (~t+10us); one early load there doubles the solo-phase
                    # load throughput without ever blocking a store
                    nc.scalar.dma_start(out=xin[:], in_=xa[pr].transpose([1, 0, 2, 3]))
                else:
                    # (t, p, c, w) -> (p, t, c, w); fully sequential DRAM read
                    nc.sync.dma_start(out=xin[:], in_=xa[pr].transpose([1, 0, 2, 3]))
                outt = opool.tile([P, 2, 2, NT, HW_OUT], f16)
                for c in range(2):
                    for g in range(NT // 2):
                        t0 = 2 * g
                        # [p, tile-pair, parity, j]
                        ev = xin[:, t0 : t0 + 2, c, :].rearrange(
                            "p t (j two) -> p t two j", two=2
                        )
                        # 2-bank PSUM tile: [0:512]=ll|lh, [512:1024]=hl|hh
                        pt = psum.tile([P, 4 * HW_OUT], f32)
                        nc.tensor.matmul(
                            pt[:, 0:512], wt[:, 0, :], ev[:, :, 0, :],
                            start=True, stop=False,
                        )
                        nc.tensor.matmul(
                            pt[:, 0:512], wt[:, 0, :], ev[:, :, 1, :],
                            start=False, stop=True,
                        )
                        nc.tensor.matmul(
                            pt[:, 512:1024], wt[:, 1, :], ev[:, :, 0, :],
                            start=True, stop=False,
                        )
                        nc.tensor.matmul(
                            pt[:, 512:1024], wt[:, 0, :], ev[:, :, 1, :],
                            start=False, stop=True,
                        )
                        # drain the two banks in parallel on ACT and DVE so
                        # PSUM frees fast and the matmul queue never stalls
                        src = pt[:].rearrange("p (b t j) -> p b t j", b=2, j=HW_OUT)
                        dst = outt[:, c, :, t0 : t0 + 2, :]
                        nc.scalar.copy(dst[:, 0], src[:, 0])
                        nc.vector.tensor_copy(dst[:, 1], src[:, 1])
                        if pr >= NPR - 2:
                            # fine-grained tail: store each group as soon as
                            # its drain lands, alternating rings — the sync
                            # ring is done with loads by now, so the tail
                            # drains on two descriptor streams
                            eng = nc.sync if (c + g) % 2 == 0 else nc.scalar
                            eng.dma_start(
                                out=oa[pr, :, c, :, t0 : t0 + 2, :],
                                in_=outt[:, c, :, t0 : t0 + 2, :],
                            )
                    if pr == 0:
                        # per-channel stores up front so the store stream
                        # joins the load stream as early as possible
                        nc.scalar.dma_start(out=oa[pr, :, c], in_=outt[:, c])
                if 0 < pr < NPR - 2:
                    # one store per pair: contiguous 8 KB per partition line.
                    # The last paired store rides the (by then idle) sync ring
                    # so the drain phase uses both rings.
                    eng = nc.sync if pr == NPR - 3 else nc.scalar
                    eng.dma_start(out=oa[pr], in_=outt[:])
    nc.compile()
    return nc


def _get_nc():
    if "nc" not in _CACHE:
        _CACHE["nc"] = _build()
    return _CACHE["nc"]


def run(x, **spmd_kwargs):
    """Run the DWT on 8 cores; returns (results_tuple, BassKernelResults)."""
    nc = _get_nc()
    # [core, pair, c, t, p, w] -> [core, pair, t, p, c, w], staged as fp16
    xs = (
        np.asarray(x, dtype=np.float32)
        .reshape(N_CORES, NPR, 2, NT, P, W)
        .astype(np.float16)
        .transpose(0, 1, 3, 4, 2, 5)
    )
    xs = np.ascontiguousarray(xs)
    wmat = _butterfly_weights().astype(np.float16)
    in_maps = [{"x": xs[i], "w": wmat} for i in range(N_CORES)]
    res = None
    for attempt in range(3):
        try:
            res = run_bass_kernel_spmd(
                nc, in_maps, core_ids=list(range(N_CORES)), **spmd_kwargs
            )
            break
        except Exception:
            # transient device wedge (NRT_EXEC_UNIT_UNRECOVERABLE) recovers
            # on retry; re-raise only if it persists
            if attempt == 2:
                raise
            import time

            time.sleep(2)
    # per-core out: (NPR, P, 2, 2, NT, HW_OUT) fp16
    full = np.stack([res.results[i]["out"] for i in range(N_CORES)])

    def expand(g, half):  # -> (B, C, 256, 256) f32
        # (cores, NPR, 64, c, NT, j) -> channel ch = 2*pr + c, row = 64*t + p64
        sl = full[:, :, 64 * half : 64 * (half + 1), :, g]
        sl = sl.transpose(0, 1, 3, 4, 2, 5)  # (cores, pr, c, t, p64, j)
        return np.ascontiguousarray(sl, dtype=np.float32).reshape(B, C, HW_OUT, HW_OUT)

    ll = expand(0, 0)
    lh = expand(0, 1)
    hl = expand(1, 0)
    hh = expand(1, 1)
    return (ll, lh, hl, hh), res


def kernel(x):
    out, _ = run(x)
    return out


# revision 39
# speedup vs baseline: 1.1051x; 1.1051x over previous
"""Haar DWT (2x2 stride-2 block decomposition) on 8 Trainium2 NeuronCores.

Input x: (32, 3, 512, 512) f32. Outputs (ll, lh, hl, hh): each (32, 3, 256, 256).

Sharding: pure data parallel over the batch dim — 4 images per core, viewed as
12 channel images of 512x512 per core.

The 2e-2 rel-err gate admits fp16 end to end (measured rel-err ~6e-4), so the
host stages the input to HBM as fp16 during sharding: HBM traffic per core is
6.29 MB in + 6.29 MB out instead of the naive 12.6 + 12.6. Channels are staged
in pairs interleaved per row ([pair, t, p, c, w]) so each DMA descriptor still
moves a contiguous 2 KB run into a partition line.

On-chip, both butterfly stages run on the TensorEngine via PSUM
accumulation — no DVE tensor-tensor work at all:

  P1 =  W @ x_even + W @ x_odd   ->  [ll (p<64) ; lh (p>=64)]
  P2 = -W @ x_even + W @ x_odd   ->  [hl ; hh]

with fp16 weights/moving operands (1 col/cycle, 4x fast weight load) and the
even/odd split taken from stride-2 access patterns. Each 2-bank PSUM group is
drained by two parallel fp32->fp16 copies (ACT and DVE halves) straight into
the store tile. Loads ride the Sync HWDGE ring, stores the ACT ring.
"""

import sys

import numpy as np

if "/opt/trn_rl_repo" not in sys.path:
    sys.path.insert(0, "/opt/trn_rl_repo")

from concourse import bacc, bass, mybir
from concourse import tile
from concourse.bass_utils import run_bass_kernel_spmd

N_CORES = 8
B, C, H, W = 32, 3, 512, 512
BPC = B // N_CORES  # images per core
NCH = BPC * C  # channel images per core (12)
NPR = NCH // 2  # channel pairs per core (6)
P = 128  # SBUF partitions
NT = H // P  # 128-row tiles per channel (4)
HW_OUT = H // 2  # 256

_CACHE = {}


def _butterfly_weights():
    """w[0] = vertical butterfly W (sum rows to p<64, diff rows to p>=64);
    w[1] = -W (for the hl/hh accumulation group's even-column pass)."""
    w = np.zeros((2, P, P), dtype=np.float32)
    for m in range(64):
        w[0, 2 * m, m] = 0.5
        w[0, 2 * m + 1, m] = 0.5
        w[0, 2 * m, 64 + m] = -0.5
        w[0, 2 * m + 1, 64 + m] = 0.5
    w[1] = -w[0]
    return w


def _build():
    nc = bacc.Bacc("TRN2", target_bir_lowering=False, debug=False)
    f32 = mybir.dt.float32
    f16 = mybir.dt.float16
    # x staged as fp16 [pair, tile, row-in-tile, channel-in-pair, W]
    x = nc.dram_tensor("x", [NPR, NT, P, 2, W], f16, kind="ExternalInput")
    w = nc.dram_tensor("w", [2, P, P], f16, kind="ExternalInput")
    # out[pair, p, c, g, t, j]: g=0: [ll (p<64) | lh (p>=64)], g=1: [hl | hh];
    # output image row r = 64*t + (p mod 64). Pair-major so one store covers
    # both channels with a contiguous 8 KB run per partition line.
    out = nc.dram_tensor(
        "out", [NPR, P, 2, 2, NT, HW_OUT], f16, kind="ExternalOutput"
    )
    xa = x.ap()
    oa = out.ap()
    with tile.TileContext(nc) as tc:
        with (
            tc.tile_pool(name="p", bufs=5) as pool,
            tc.tile_pool(name="o", bufs=4) as opool,
            tc.tile_pool(name="w", bufs=1) as wpool,
            tc.tile_pool(name="ps", bufs=4, space=bass.MemorySpace.PSUM) as psum,
        ):
            wt = wpool.tile([P, 2, P], f16)
            for pr in range(NPR):
                xin = pool.tile([P, NT, 2, W], f16)
                if pr == 0 or pr == NPR - 1:
                    # split the first load (matmuls start early) and the last
                    # load (the tail pair's matmuls aren't gated on the full
                    # 1 MiB transfer). For pair 0, order the queue by what
                    # gates the first matmul group: tiles 0-1, weights, 2-3.
                    for t in range(2):
                        nc.sync.dma_start(out=xin[:, t], in_=xa[pr, t])
                    if pr == 0:
                        nc.sync.dma_start(out=wt[:], in_=w.ap().transpose([1, 0, 2]))
                    for t in range(2, NT):
                        nc.sync.dma_start(out=xin[:, t], in_=xa[pr, t])
                elif pr == 1:
                    # the scalar HWDGE ring is idle until the first store
                    # (~t+10us); one early load there doubles the solo-phase
                    # load throughput without ever blocking a store
                    nc.scalar.dma_start(out=xin[:], in_=xa[pr].transpose([1, 0, 2, 3]))
                else:
                    # (t, p, c, w) -> (p, t, c, w); fully sequential DRAM read
                    nc.sync.dma_start(out=xin[:], in_=xa[pr].transpose([1, 0, 2, 3]))
                outt = opool.tile([P, 2, 2, NT, HW_OUT], f16)
                for c in range(2):
                    for g in range(NT // 2):
                        t0 = 2 * g
                        # [p, tile-pair, parity, j]
                        ev = xin[:, t0 : t0 + 2, c, :].rearrange(
                            "p t (j two) -> p t two j", two=2
                        )
                        # 2-bank PSUM tile: [0:512]=ll|lh, [512:1024]=hl|hh
                        pt = psum.tile([P, 4 * HW_OUT], f32)
                        nc.tensor.matmul(
                            pt[:, 0:512], wt[:, 0, :], ev[:, :, 0, :],
                            start=True, stop=False,
                        )
                        nc.tensor.matmul(
                            pt[:, 0:512], wt[:, 0, :], ev[:, :, 1, :],
                            start=False, stop=True,
                        )
                        nc.tensor.matmul(
                            pt[:, 512:1024], wt[:, 1, :], ev[:, :, 0, :],
                            start=True, stop=False,
                        )
                        nc.tensor.matmul(
                            pt[:, 512:1024], wt[:, 0, :], ev[:, :, 1, :],
                            start=False, stop=True,
                        )
                        # drain the two banks in parallel on ACT and DVE so
                        # PSUM frees fast and the matmul queue never stalls
                        src = pt[:].rearrange("p (b t j) -> p b t j", b=2, j=HW_OUT)
                        dst = outt[:, c, :, t0 : t0 + 2, :]
                        nc.scalar.copy(dst[:, 0], src[:, 0])
                        nc.vector.tensor_copy(dst[:, 1], src[:, 1])
                        if pr >= NPR - 2:
                            # fine-grained tail: store each group as soon as
                            # its drain lands so the pipeline flushes quickly
                            nc.scalar.dma_start(
                                out=oa[pr, :, c, :, t0 : t0 + 2, :],
                                in_=outt[:, c, :, t0 : t0 + 2, :],
                            )
                    if pr == 0:
                        # per-channel stores up front so the store stream
                        # joins the load stream as early as possible
                        nc.scalar.dma_start(out=oa[pr, :, c], in_=outt[:, c])
                if 0 < pr < NPR - 2:
                    # one store per pair: contiguous 8 KB per partition line
                    nc.scalar.dma_start(out=oa[pr], in_=outt[:])
    nc.compile()
    return nc


def _get_nc():
    if "nc" not in _CACHE:
        _CACHE["nc"] = _build()
    return _CACHE["nc"]


def run(x, **spmd_kwargs):
    """Run the DWT on 8 cores; returns (results_tuple, BassKernelResults)."""
    nc = _get_nc()
    # [core, pair, c, t, p, w] -> [core, pair, t, p, c, w], staged as fp16
    xs = (
        np.asarray(x, dtype=np.float32)
        .reshape(N_CORES, NPR, 2, NT, P, W)
        .astype(np.float16)
        .transpose(0, 1, 3, 4, 2, 5)
    )
    xs = np.ascontiguousarray(xs)
    wmat = _butterfly_weights().astype(np.float16)
    in_maps = [{"x": xs[i], "w": wmat} for i in range(N_CORES)]
    res = None
    for attempt in range(3):
        try:
            res = run_bass_kernel_spmd(
                nc, in_maps, core_ids=list(range(N_CORES)), **spmd_kwargs
            )
            break
        except Exception:
            # transient device wedge (NRT_EXEC_UNIT_UNRECOVERABLE) recovers
            # on retry; re-raise only if it persists
            if attempt == 2:
                raise
            import time

            time.sleep(2)
    # per-core out: (NPR, P, 2, 2, NT, HW_OUT) fp16
    full = np.stack([res.results[i]["out"] for i in range(N_CORES)])

    def expand(g, half):  # -> (B, C, 256, 256) f32
        # (cores, NPR, 64, c, NT, j) -> channel ch = 2*pr + c, row = 64*t + p64
        sl = full[:, :, 64 * half : 64 * (half + 1), :, g]
        sl = sl.transpose(0, 1, 3, 4, 2, 5)  # (cores, pr, c, t, p64, j)
        return np.ascontiguousarray(sl, dtype=np.float32).reshape(B, C, HW_OUT, HW_OUT)

    ll = expand(0, 0)
    lh = expand(0, 1)
    hl = expand(1, 0)
    hh = expand(1, 1)
    return (ll, lh, hl, hh), res


def kernel(x):
    out, _ = run(x)
    return out
